# revision 13
# baseline (speedup 1.0000x reference)
"""Trainium2 Bass kernel for CareerTreeModel (2x SAGEConv + BN + edge-pair MLP).

Three SPMD launches over 8 NeuronCores (host concatenates shard outputs
between launches; the axon runtime's collectives are not functional):
  A: conv1 -- indirect-DMA edge gather of x[src], one-hot segment-sum matmul
     into PSUM (feature-major), BN folded into weights, h and hl = h@c2_Wl.T
     produced per-core.
  B: conv2 -- same aggregation over the host-gathered hl table, + h@c2_Wr.T,
     z produced node-major per-core.
  C: edge-pair MLP -- gather z[i]|z[j] per 128-pair tile, PE-transpose to
     feature-major, 3 matmuls + relu/relu/sigmoid ACT epilogues.
"""

import sys

if "/opt/trn_rl_repo" not in sys.path:
    sys.path.insert(0, "/opt/trn_rl_repo")

import time as _time

import numpy as np
import ml_dtypes

import concourse.bass as bass
import concourse.bacc as bacc
import concourse.mybir as mybir
import concourse.tile as tile
from concourse import bass_utils

BF16 = ml_dtypes.bfloat16

N = 50000
E = 800000
C = 8
NLOC = N // C
SBN = 256
NSB = 25
NP_LOC = NSB * SBN       # 6400
NPG = C * NP_LOC         # 51200
K_REG = 34
K_LAST = 15
K_PROG = [K_REG] * 24 + [K_LAST]
PAD_SLOTS = sum(K_PROG) * 128
PAIRS_PC = E // C
PGROUPS = 196
PAIR_PAD = PGROUPS * 512
D_IN = 128
D_OUT = 64
BN_EPS = 1e-5

_PROG = {}
_LAST_RES = None
_LAST_EXEC_NS = None

f32 = mybir.dt.float32
bf16 = mybir.dt.bfloat16
i32 = mybir.dt.int32


def _build_A():
    nc = bacc.Bacc("TRN2", target_bir_lowering=False, debug=False,
                   enable_asserts=False, num_devices=C)
    t_x = nc.dram_tensor("x_full", [N, D_IN], bf16, kind="ExternalInput")
    t_xT = nc.dram_tensor("x_own_T", [128, NP_LOC], bf16, kind="ExternalInput")
    t_si1 = nc.dram_tensor("sidx1", [NSB * 128, K_REG], i32, kind="ExternalInput")
    t_dstr = nc.dram_tensor("dstrel", [NSB * 128, K_REG], f32, kind="ExternalInput")
    t_invb = nc.dram_tensor("invb", [NSB, SBN], f32, kind="ExternalInput")
    t_iota = nc.dram_tensor("iota", [128, SBN], bf16, kind="ExternalInput")
    t_w1l = nc.dram_tensor("c1WlT", [128, 128], bf16, kind="ExternalInput")
    t_w1r = nc.dram_tensor("c1WrT", [128, 128], bf16, kind="ExternalInput")
    t_b1c = nc.dram_tensor("b1c", [128, 1], f32, kind="ExternalInput")
    t_w2l = nc.dram_tensor("c2WlT", [128, 64], bf16, kind="ExternalInput")
    t_h = nc.dram_tensor("h_out", [128, NP_LOC], bf16, kind="ExternalOutput")
    t_hl = nc.dram_tensor("hl_out", [NP_LOC, D_OUT], bf16, kind="ExternalOutput")

    with tile.TileContext(nc) as tc:
        with tc.tile_pool(name="consts", bufs=1) as cp, \
             tc.tile_pool(name="work", bufs=2) as wp, \
             tc.tile_pool(name="sb3", bufs=3) as wp3, \
             tc.tile_pool(name="psA", bufs=2, space="PSUM") as psA, \
             tc.tile_pool(name="psM", bufs=3, space="PSUM") as psM:
            iota = cp.tile([128, SBN], bf16, tag="c_iota")
            nc.sync.dma_start(out=iota[:], in_=t_iota.ap())
            w1l = cp.tile([128, 128], bf16, tag="c_w1l")
            nc.sync.dma_start(out=w1l[:], in_=t_w1l.ap())
            w1r = cp.tile([128, 128], bf16, tag="c_w1r")
            nc.sync.dma_start(out=w1r[:], in_=t_w1r.ap())
            b1c = cp.tile([128, 1], f32, tag="c_b1c")
            nc.sync.dma_start(out=b1c[:], in_=t_b1c.ap())
            w2l = cp.tile([128, 64], bf16, tag="c_w2l")
            nc.sync.dma_start(out=w2l[:], in_=t_w2l.ap())

            for sb in range(NSB):
                K = K_PROG[sb]
                r0 = sb * 128
                idx = wp.tile([128, K_REG], i32, tag="idx1")
                nc.sync.dma_start(out=idx[:], in_=t_si1.ap()[r0:r0 + 128, :])
                dstr = wp.tile([128, K_REG], f32, tag="dstr")
                nc.sync.dma_start(out=dstr[:], in_=t_dstr.ap()[r0:r0 + 128, :])
                invb = wp.tile([128, SBN], f32, tag="invb")
                nc.sync.dma_start(
                    out=invb[:],
                    in_=t_invb.ap()[sb:sb + 1, :].to_broadcast((128, SBN)))
                msgs = wp.tile([128, K_REG, D_IN], bf16, tag="msgs")
                agg = psA.tile([128, SBN], f32, tag="agg", space="PSUM")
                for j in range(K):
                    nc.gpsimd.indirect_dma_start(
                        out=msgs[:, j, :], out_offset=None, in_=t_x.ap(),
                        in_offset=bass.IndirectOffsetOnAxis(ap=idx[:, j:j + 1], axis=0),
                    )
                    S = wp3.tile([128, SBN], bf16, tag="S")
                    nc.vector.tensor_scalar(
                        out=S[:], in0=iota[:], scalar1=dstr[:, j:j + 1], scalar2=None,
                        op0=mybir.AluOpType.is_equal)
                    nc.tensor.matmul(out=agg[:], lhsT=msgs[:, j, :], rhs=S[:],
                                     start=(j == 0), stop=(j == K - 1))
                mean = wp.tile([128, SBN], bf16, tag="mean")
                nc.vector.tensor_tensor(out=mean[:], in0=agg[:], in1=invb[:],
                                        op=mybir.AluOpType.mult)
                xT = wp.tile([128, SBN], bf16, tag="xT")
                nc.sync.dma_start(out=xT[:], in_=t_xT.ap()[:, sb * SBN:(sb + 1) * SBN])
                hpre = psA.tile([128, SBN], f32, tag="agg", space="PSUM")
                nc.tensor.matmul(out=hpre[:], lhsT=w1l[:], rhs=mean[:],
                                 start=True, stop=False)
                nc.tensor.matmul(out=hpre[:], lhsT=w1r[:], rhs=xT[:],
                                 start=False, stop=True)
                hT = wp.tile([128, SBN], bf16, tag="hT")
                nc.scalar.activation(out=hT[:], in_=hpre[:],
                                     func=mybir.ActivationFunctionType.Relu,
                                     bias=b1c[:], scale=1.0)
                nc.sync.dma_start(out=t_h.ap()[:, sb * SBN:(sb + 1) * SBN], in_=hT[:])
                for half in range(2):
                    n0 = sb * SBN + half * 128
                    hlp = psM.tile([128, 64], f32, tag="mlp", space="PSUM")
                    nc.tensor.matmul(out=hlp[:], lhsT=hT[:, half * 128:(half + 1) * 128],
                                     rhs=w2l[:], start=True, stop=True)
                    hl_sb = wp.tile([128, D_OUT], bf16, tag="hl")
                    nc.scalar.activation(out=hl_sb[:], in_=hlp[:],
                                         func=mybir.ActivationFunctionType.Copy,
                                         scale=1.0)
                    nc.sync.dma_start(out=t_hl.ap()[n0:n0 + 128, :], in_=hl_sb[:])
    nc.compile()
    return nc


def _build_B():
    nc = bacc.Bacc("TRN2", target_bir_lowering=False, debug=False,
                   enable_asserts=False, num_devices=C)
    t_hlf = nc.dram_tensor("hl_full", [NPG, D_OUT], bf16, kind="ExternalInput")
    t_hT = nc.dram_tensor("h_in", [128, NP_LOC], bf16, kind="ExternalInput")
    t_si2 = nc.dram_tensor("sidx2", [NSB * 128, K_REG], i32, kind="ExternalInput")
    t_dstr = nc.dram_tensor("dstrel", [NSB * 128, K_REG], f32, kind="ExternalInput")
    t_invb = nc.dram_tensor("invb", [NSB, SBN], f32, kind="ExternalInput")
    t_iota = nc.dram_tensor("iota", [128, SBN], bf16, kind="ExternalInput")
    t_ident = nc.dram_tensor("ident", [128, 128], bf16, kind="ExternalInput")
    t_w2r = nc.dram_tensor("c2WrT", [128, 64], bf16, kind="ExternalInput")
    t_z = nc.dram_tensor("z_out", [NP_LOC, D_OUT], bf16, kind="ExternalOutput")

    with tile.TileContext(nc) as tc:
        with tc.tile_pool(name="consts", bufs=1) as cp, \
             tc.tile_pool(name="work", bufs=2) as wp, \
             tc.tile_pool(name="sb3", bufs=3) as wp3, \
             tc.tile_pool(name="psA", bufs=2, space="PSUM") as psA, \
             tc.tile_pool(name="psT", bufs=2, space="PSUM") as psT:
            iota = cp.tile([128, SBN], bf16, tag="c_iota")
            nc.sync.dma_start(out=iota[:], in_=t_iota.ap())
            ident = cp.tile([128, 128], bf16, tag="c_ident")
            nc.sync.dma_start(out=ident[:], in_=t_ident.ap())
            w2r = cp.tile([128, 64], bf16, tag="c_w2r")
            nc.sync.dma_start(out=w2r[:], in_=t_w2r.ap())
            hres = cp.tile([128, NP_LOC], bf16, tag="c_h")
            nc.sync.dma_start(out=hres[:], in_=t_hT.ap())

            for sb in range(NSB):
                K = K_PROG[sb]
                r0 = sb * 128
                idx = wp.tile([128, K_REG], i32, tag="idx1")
                nc.sync.dma_start(out=idx[:], in_=t_si2.ap()[r0:r0 + 128, :])
                dstr = wp.tile([128, K_REG], f32, tag="dstr")
                nc.sync.dma_start(out=dstr[:], in_=t_dstr.ap()[r0:r0 + 128, :])
                invb = wp.tile([128, SBN], f32, tag="invb")
                nc.sync.dma_start(
                    out=invb[:],
                    in_=t_invb.ap()[sb:sb + 1, :].to_broadcast((128, SBN)))
                msgs2 = wp.tile([128, K_REG, D_OUT], bf16, tag="msgs2")
                agg2 = psA.tile([64, SBN], f32, tag="agg", space="PSUM")
                for j in range(K):
                    nc.gpsimd.indirect_dma_start(
                        out=msgs2[:, j, :], out_offset=None, in_=t_hlf.ap(),
                        in_offset=bass.IndirectOffsetOnAxis(ap=idx[:, j:j + 1], axis=0),
                    )
                    S = wp3.tile([128, SBN], bf16, tag="S")
                    nc.vector.tensor_scalar(
                        out=S[:], in0=iota[:], scalar1=dstr[:, j:j + 1], scalar2=None,
                        op0=mybir.AluOpType.is_equal)
                    nc.tensor.matmul(out=agg2[:], lhsT=msgs2[:, j, :], rhs=S[:],
                                     start=(j == 0), stop=(j == K - 1))
                hrp = psA.tile([64, SBN], f32, tag="agg", space="PSUM")
                nc.tensor.matmul(out=hrp[:], lhsT=w2r[:],
                                 rhs=hres[:, sb * SBN:(sb + 1) * SBN],
                                 start=True, stop=True)
                zt = wp.tile([64, SBN], f32, tag="zt")
                nc.vector.tensor_tensor(out=zt[:], in0=agg2[:], in1=invb[:64, :],
                                        op=mybir.AluOpType.mult)
                zbf = wp.tile([64, SBN], bf16, tag="zbf")
                nc.vector.tensor_tensor(out=zbf[:], in0=zt[:], in1=hrp[:],
                                        op=mybir.AluOpType.add)
                for half in range(2):
                    n0 = sb * SBN + half * 128
                    trp = psT.tile([128, 128], bf16, tag="trb", space="PSUM")
                    nc.tensor.transpose(out=trp[:, :D_OUT],
                                        in_=zbf[:, half * 128:(half + 1) * 128],
                                        identity=ident[:64, :64])
                    z_sb = wp.tile([128, D_OUT], bf16, tag="hl")
                    nc.scalar.activation(out=z_sb[:], in_=trp[:, :D_OUT],
                                         func=mybir.ActivationFunctionType.Copy,
                                         scale=1.0)
                    nc.sync.dma_start(out=t_z.ap()[n0:n0 + 128, :], in_=z_sb[:])
    nc.compile()
    return nc


def _build_C():
    nc = bacc.Bacc("TRN2", target_bir_lowering=False, debug=False,
                   enable_asserts=False, num_devices=C)
    t_zf = nc.dram_tensor("z_full", [NPG, D_OUT], bf16, kind="ExternalInput")
    t_pi = nc.dram_tensor("pi_cols", [128, PAIR_PAD // 128], i32, kind="ExternalInput")
    t_pj = nc.dram_tensor("pj_cols", [128, PAIR_PAD // 128], i32, kind="ExternalInput")
    t_ident = nc.dram_tensor("ident", [128, 128], bf16, kind="ExternalInput")
    t_ew1 = nc.dram_tensor("eW1T", [128, 64], bf16, kind="ExternalInput")
    t_eb1 = nc.dram_tensor("eb1", [64, 1], f32, kind="ExternalInput")
    t_ew2 = nc.dram_tensor("eW2T", [64, 32], bf16, kind="ExternalInput")
    t_eb2 = nc.dram_tensor("eb2", [32, 1], f32, kind="ExternalInput")
    t_ew3 = nc.dram_tensor("eW3T", [32, 1], bf16, kind="ExternalInput")
    t_eb3 = nc.dram_tensor("eb3", [1, 1], f32, kind="ExternalInput")
    t_out = nc.dram_tensor("out", [PGROUPS, 512], f32, kind="ExternalOutput")

    with tile.TileContext(nc) as tc:
        with tc.tile_pool(name="consts", bufs=1) as cp, \
             tc.tile_pool(name="work", bufs=2) as wp, \
             tc.tile_pool(name="sb3", bufs=3) as wp3, \
             tc.tile_pool(name="psT", bufs=2, space="PSUM") as psT, \
             tc.tile_pool(name="psM", bufs=3, space="PSUM") as psM:
            ident = cp.tile([128, 128], bf16, tag="c_ident")
            nc.sync.dma_start(out=ident[:], in_=t_ident.ap())
            ew1 = cp.tile([128, 64], bf16, tag="c_ew1")
            nc.sync.dma_start(out=ew1[:], in_=t_ew1.ap())
            eb1 = cp.tile([64, 1], f32, tag="c_eb1")
            nc.sync.dma_start(out=eb1[:], in_=t_eb1.ap())
            ew2 = cp.tile([64, 32], bf16, tag="c_ew2")
            nc.sync.dma_start(out=ew2[:], in_=t_ew2.ap())
            eb2 = cp.tile([32, 1], f32, tag="c_eb2")
            nc.sync.dma_start(out=eb2[:], in_=t_eb2.ap())
            ew3 = cp.tile([32, 1], bf16, tag="c_ew3")
            nc.sync.dma_start(out=ew3[:], in_=t_ew3.ap())
            eb3 = cp.tile([1, 1], f32, tag="c_eb3")
            nc.sync.dma_start(out=eb3[:], in_=t_eb3.ap())
            pit = cp.tile([128, PAIR_PAD // 128], i32, tag="c_pi")
            nc.sync.dma_start(out=pit[:], in_=t_pi.ap())
            pjt = cp.tile([128, PAIR_PAD // 128], i32, tag="c_pj")
            nc.sync.dma_start(out=pjt[:], in_=t_pj.ap())

            for g in range(PGROUPS):
                catT = wp.tile([128, 512], bf16, tag="catT")
                for t in range(4):
                    ti = g * 4 + t
                    cat = wp3.tile([128, 128], bf16, tag="cat")
                    nc.gpsimd.indirect_dma_start(
                        out=cat[:, 0:64], out_offset=None, in_=t_zf.ap(),
                        in_offset=bass.IndirectOffsetOnAxis(ap=pit[:, ti:ti + 1], axis=0),
                    )
                    nc.gpsimd.indirect_dma_start(
                        out=cat[:, 64:128], out_offset=None, in_=t_zf.ap(),
                        in_offset=bass.IndirectOffsetOnAxis(ap=pjt[:, ti:ti + 1], axis=0),
                    )
                    trp = psT.tile([128, 128], bf16, tag="trb", space="PSUM")
                    nc.tensor.transpose(out=trp[:], in_=cat[:], identity=ident[:])
                    nc.scalar.activation(out=catT[:, t * 128:(t + 1) * 128], in_=trp[:],
                                         func=mybir.ActivationFunctionType.Copy,
                                         scale=1.0)
                e1p = psM.tile([64, 512], f32, tag="mlp", space="PSUM")
                nc.tensor.matmul(out=e1p[:], lhsT=ew1[:], rhs=catT[:],
                                 start=True, stop=True)
                e1 = wp.tile([64, 512], bf16, tag="e1")
                nc.scalar.activation(out=e1[:], in_=e1p[:],
                                     func=mybir.ActivationFunctionType.Relu,
                                     bias=eb1[:], scale=1.0)
                e2p = psM.tile([32, 512], f32, tag="mlp", space="PSUM")
                nc.tensor.matmul(out=e2p[:], lhsT=ew2[:], rhs=e1[:],
                                 start=True, stop=True)
                e2 = wp.tile([32, 512], bf16, tag="e2")
                nc.scalar.activation(out=e2[:], in_=e2p[:],
                                     func=mybir.ActivationFunctionType.Relu,
                                     bias=eb2[:], scale=1.0)
                e3p = psM.tile([1, 512], f32, tag="mlp", space="PSUM")
                nc.tensor.matmul(out=e3p[:], lhsT=ew3[:], rhs=e2[:],
                                 start=True, stop=True)
                orow = wp3.tile([1, 512], f32, tag="orow")
                nc.scalar.activation(out=orow[:], in_=e3p[:],
                                     func=mybir.ActivationFunctionType.Sigmoid,
                                     bias=eb3[:], scale=1.0)
                nc.sync.dma_start(out=t_out.ap()[g:g + 1, :], in_=orow[:])
    nc.compile()
    return nc


def _gpad(v):
    return (v // NLOC) * NP_LOC + (v % NLOC)


def kernel(**inputs):
    global _LAST_RES
    x = np.asarray(inputs["x"], np.float32)
    ei = np.asarray(inputs["edge_index"]).astype(np.int64)
    ep = np.asarray(inputs["edge_pairs"]).astype(np.int64)
    src, dst = ei[0], ei[1]

    own = dst // NLOC
    loc = dst % NLOC
    sbi = loc // SBN
    order = np.lexsort((dst, sbi, own))
    s_src, s_dst, s_own, s_sbi = src[order], dst[order], own[order], sbi[order]
    s_rel = (s_dst % NLOC) % SBN + 0.0

    grp = s_own * NSB + s_sbi
    cnt = np.bincount(grp, minlength=C * NSB).reshape(C, NSB)
    Kneed = np.ceil(cnt / 128).astype(int)
    assert (Kneed[:, :24] <= K_REG).all() and (Kneed[:, 24] <= K_LAST).all(), Kneed.max()

    sb_off = np.concatenate(([0], np.cumsum(np.array(K_PROG) * 128)))[:-1]
    gstart = np.concatenate(([0], np.cumsum(cnt.ravel())))[:-1].reshape(C, NSB)
    rank = np.arange(E) - gstart[s_own, s_sbi]
    slot = s_own * PAD_SLOTS + sb_off[s_sbi] + rank

    sidx1 = np.zeros(C * PAD_SLOTS, np.int32)
    sidx2 = np.zeros(C * PAD_SLOTS, np.int32)
    dstrel = np.full(C * PAD_SLOTS, -1.0, np.float32)
    sidx1[slot] = s_src
    sidx2[slot] = _gpad(s_src)
    dstrel[slot] = s_rel
    sidx1 = sidx1.reshape(C, PAD_SLOTS)
    sidx2 = sidx2.reshape(C, PAD_SLOTS)
    dstrel = dstrel.reshape(C, PAD_SLOTS)

    def to_dev(a, dt):
        out = np.zeros((C, NSB * 128, K_REG), dt)
        for sb in range(NSB):
            K = K_PROG[sb]
            blk = a[:, sb_off[sb]:sb_off[sb] + K * 128].reshape(C, K, 128)
            out[:, sb * 128:(sb + 1) * 128, :K] = blk.transpose(0, 2, 1)
        return out

    sidx1_d = to_dev(sidx1, np.int32)
    sidx2_d = to_dev(sidx2, np.int32)
    dstr_d = to_dev(dstrel, np.float32)

    indeg = np.bincount(dst, minlength=N).astype(np.float32)
    inv = 1.0 / np.maximum(indeg, 1.0)
    inv_pad = np.zeros((C, NP_LOC), np.float32)
    inv_pad[:, :NLOC] = inv.reshape(C, NLOC)
    invb_d = inv_pad.reshape(C, NSB, SBN).copy()

    pi_cols = np.zeros((C, 128, PAIR_PAD // 128), np.int32)
    pj_cols = np.zeros((C, 128, PAIR_PAD // 128), np.int32)
    for c in range(C):
        p0 = np.zeros(PAIR_PAD, np.int64)
        p1 = np.zeros(PAIR_PAD, np.int64)
        p0[:PAIRS_PC] = ep[0, c * PAIRS_PC:(c + 1) * PAIRS_PC]
        p1[:PAIRS_PC] = ep[1, c * PAIRS_PC:(c + 1) * PAIRS_PC]
        pi_cols[c] = _gpad(p0).reshape(-1, 128).T
        pj_cols[c] = _gpad(p1).reshape(-1, 128).T

    x_bf = x.astype(BF16)
    xT_pad = np.zeros((C, 128, NP_LOC), BF16)
    for c in range(C):
        xT_pad[c, :, :NLOC] = x[c * NLOC:(c + 1) * NLOC].T.astype(BF16)

    sc = (np.asarray(inputs["bn_gamma"], np.float32)
          / np.sqrt(np.asarray(inputs["bn_var"], np.float32) + BN_EPS))
    c1Wl = np.asarray(inputs["c1_Wl"], np.float32) * sc[:, None]
    c1Wr = np.asarray(inputs["c1_Wr"], np.float32) * sc[:, None]
    b1c = (np.asarray(inputs["bn_beta"], np.float32)
           + sc * (np.asarray(inputs["c1_bl"], np.float32)
                   - np.asarray(inputs["bn_mean"], np.float32)))
    eW1 = np.asarray(inputs["e_W1"], np.float32)
    eb1 = (np.asarray(inputs["e_b1"], np.float32)
           + (eW1[:, :D_OUT] + eW1[:, D_OUT:]) @ np.asarray(inputs["c2_bl"], np.float32))

    iota_h = np.tile(np.arange(SBN, dtype=np.float32)[None, :], (128, 1)).astype(BF16)
    ident_h = np.eye(128, dtype=np.float32).astype(BF16)

    if "A" not in _PROG:
        _PROG["A"] = _build_A()
    if "B" not in _PROG:
        _PROG["B"] = _build_B()
    if "C" not in _PROG:
        _PROG["C"] = _build_C()

    cores = list(range(C))

    in_A = []
    for c in range(C):
        in_A.append({
            "x_full": x_bf, "x_own_T": xT_pad[c], "sidx1": sidx1_d[c],
            "dstrel": dstr_d[c], "invb": invb_d[c], "iota": iota_h,
            "c1WlT": c1Wl.T.copy().astype(BF16),
            "c1WrT": c1Wr.T.copy().astype(BF16),
            "b1c": b1c[:, None].copy(),
            "c2WlT": np.asarray(inputs["c2_Wl"], np.float32).T.copy().astype(BF16),
        })
    _t0 = _time.time()
    resA = bass_utils.run_bass_kernel_spmd(_PROG["A"], in_A, core_ids=cores)
    _tA = _time.time() - _t0
    hl_full = np.concatenate([resA.results[c]["hl_out"] for c in range(C)], axis=0)

    in_B = []
    for c in range(C):
        in_B.append({
            "hl_full": hl_full, "h_in": resA.results[c]["h_out"],
            "sidx2": sidx2_d[c], "dstrel": dstr_d[c], "invb": invb_d[c],
            "iota": iota_h, "ident": ident_h,
            "c2WrT": np.asarray(inputs["c2_Wr"], np.float32).T.copy().astype(BF16),
        })
    _t0 = _time.time()
    resB = bass_utils.run_bass_kernel_spmd(_PROG["B"], in_B, core_ids=cores)
    _tB = _time.time() - _t0
    z_full = np.concatenate([resB.results[c]["z_out"] for c in range(C)], axis=0)

    in_C = []
    for c in range(C):
        in_C.append({
            "z_full": z_full, "pi_cols": pi_cols[c], "pj_cols": pj_cols[c],
            "ident": ident_h,
            "eW1T": eW1.T.copy().astype(BF16), "eb1": eb1[:, None].copy(),
            "eW2T": np.asarray(inputs["e_W2"], np.float32).T.copy().astype(BF16),
            "eb2": np.asarray(inputs["e_b2"], np.float32)[:, None].copy(),
            "eW3T": np.asarray(inputs["e_W3"], np.float32).T.copy().astype(BF16),
            "eb3": np.asarray(inputs["e_b3"], np.float32)[:, None].copy(),
        })
    _t0 = _time.time()
    resC = bass_utils.run_bass_kernel_spmd(_PROG["C"], in_C, core_ids=cores)
    _tC = _time.time() - _t0
    _LAST_RES = (resA, resB, resC)
    global _LAST_EXEC_NS
    times = [r.exec_time_ns for r in (resA, resB, resC)]
    if any(times):
        _LAST_EXEC_NS = sum(t for t in times if t)
    else:
        _LAST_EXEC_NS = int((_tA + _tB + _tC) * 1e9)  # wall upper bound
    sys.modules[__name__]._PER_LAUNCH_NS = times if any(times) else [
        int(_tA * 1e9), int(_tB * 1e9), int(_tC * 1e9)]

    out = np.empty((E, 1), np.float32)
    for c in range(C):
        out[c * PAIRS_PC:(c + 1) * PAIRS_PC, 0] = \
            resC.results[c]["out"].ravel()[:PAIRS_PC]
    return out
